# revision 32
# baseline (speedup 1.0000x reference)
"""Trainium2 Bass kernel for causal self-attention (B=4, T=2048, D=1024, H=16).

Sharding: 8 cores = 4 batches x 2 head-groups (data + head/tensor parallel).
Core (b, r) computes, for batch b, the Q/K/V projections of heads 8r..8r+7
over the full sequence, causal attention for those heads, and the PARTIAL
output projection y_hg @ w_out[hg-rows] for all 2048 tokens. The host sums
the two partials per batch (the "all-reduce after out_proj" of the sharding
hint, done host-side on the fp32 partials) - no device collectives and no
duplicated projection work.

All matmuls bf16 with fp32 PSUM accumulation. x is sent pre-transposed.

The kernel is scheduled for a gapless PE instruction stream: on this part the
PE clock reaches 2.4 GHz only after ~3us of continuous busy time and drops to
1.2 GHz after every idle gap (measured), so projection / attention / output-
projection work is interleaved at ~1-2us granularity rather than phased.
Attention runs in two query-range passes (q-blocks 0-7, then 8-15) to bound
exp-buffer SBUF liveness. Most projections issue during pass 1 between QK
chunk groups; V-rows 8-15, the late Q projections, the pass-1 output
projection, and the trailing pairs' AV fill the ACT(exp)-bound stretches of
pass 2. QK processes the two heads of a pair on disjoint PE row groups
(interleaved per 512-col piece, ~2x overlap), exp runs on up-to-1024-col PSUM
tiles to amortize the ~180ns/instr ACT overhead, and AV uses the [q, dh]
orientation with a ones-column in V so the softmax denominator lands as a
per-partition scalar (cheap DVE reciprocal + scalar multiply).
"""

import numpy as np
import ml_dtypes

import concourse.bass as bass
import concourse.tile as tile
from concourse import mybir
from concourse.bass_utils import run_bass_kernel_spmd

P = 128
T = 2048
D = 1024
H = 16
DH = 64
NPAIR = 4          # head pairs per core (8 heads)
NCH = 16           # key chunks of 128
BF16 = mybir.dt.bfloat16
F32 = mybir.dt.float32

# pass boundary: q-blocks 0..QS-1 in pass 1, QS..15 in pass 2 (QS chosen so
# the exp work is balanced across the two passes)
QS = 10
W1 = [(QS - c) * P for c in range(QS)]
OFF1 = np.cumsum([0] + W1).tolist()            # total 7040
W2 = [(16 - max(c, QS)) * P for c in range(NCH)]
OFF2 = np.cumsum([0] + W2).tolist()            # total 10368

_CACHED_NC = None
LAST_RESULTS = None
DEBUG = False


def _build_nc():
    nc = bass.Bass()
    xT = nc.declare_dram_parameter("xT", [D, T], BF16, isOutput=False)
    wq = nc.declare_dram_parameter("wq", [D, 512], BF16, isOutput=False)
    wk = nc.declare_dram_parameter("wk", [D, 512], BF16, isOutput=False)
    wv = nc.declare_dram_parameter("wv", [D, 512], BF16, isOutput=False)
    w_out = nc.declare_dram_parameter("w_out", [512, D], BF16, isOutput=False)
    mask = nc.declare_dram_parameter("mask", [P, P], BF16, isOutput=False)
    out = nc.declare_dram_parameter("out", [T, D], F32, isOutput=True)

    with tile.TileContext(nc) as tc:
        with (
            tc.tile_pool(name="persist", bufs=1) as pp,
            tc.tile_pool(name="qkps", bufs=2, space="PSUM") as qkp,
            tc.tile_pool(name="smallps", bufs=2, space="PSUM") as sps,
            tc.tile_pool(name="projps", bufs=2, space="PSUM") as pjp,
        ):
            kT = pp.tile([P, NPAIR, T], BF16)        # K^T, pair-dh on partitions
            qT_b = pp.tile([P, NPAIR, 1024], BF16)   # Q^T, q 1024:2048
            vA = pp.tile([P, NCH, 8, DH + 1], BF16)  # V + ones column per head
            yT = pp.tile([P, NPAIR, T], BF16)        # normalized attn out, T
            msk = pp.tile([P, P], BF16)
            wo_s = pp.tile([P, 4, D], BF16)
            nc.gpsimd.memset(vA[:, :, :, DH], 1.0)

            # mutable refs filled per scope
            box = {}

            def proj_kq(ws, dst_ap, xs, xoff):
                ps = pjp.tile([P, 512], F32, tag="pj")
                for dc in range(8):
                    nc.tensor.matmul(ps[:], ws[:, dc, :],
                                     xs[:, dc, xoff:xoff + 512],
                                     start=(dc == 0), stop=(dc == 7))
                nc.vector.tensor_copy(out=dst_ap, in_=ps[:])

            def proj_v(tt):
                xs = box['xTs_a'] if tt < 8 else box['xTs_b']
                xoff = (tt % 8) * P
                ps = pjp.tile([P, 512], F32, tag="pj")
                for dc in range(8):
                    nc.tensor.matmul(ps[:], xs[:, dc, xoff:xoff + P],
                                     box['wv_s'][:, dc, :],
                                     start=(dc == 0), stop=(dc == 7))
                nc.vector.tensor_copy(
                    out=vA[:, tt, :, 0:DH],
                    in_=ps.rearrange("p (h d) -> p h d", d=DH))

            def qk_chunk(ehs, p, c, npass):
                """QK + exp + mask for chunk c of pair p, two heads on disjoint
                PE row groups, interleaved per 512-col piece. The q-span is cut
                into <=1024-col PSUM tile pieces (one exp instruction each);
                matmul sub-pieces additionally split at PSUM bank boundaries
                and at the qT_a/qT_b tile boundary (global q col 1024)."""
                if npass == 1:
                    width, off, qg0 = W1[c], OFF1[c], c * P
                else:
                    width, off, qg0 = W2[c], OFF2[c], max(c, QS) * P
                for ps0 in range(0, width, 1024):
                    pw = min(1024, width - ps0)
                    tiles = [qkp.tile([P, 1024], F32, tag="qk",
                                      name=f"qk{npass}_{p}_{c}_{ps0}a"),
                             qkp.tile([P, 1024], F32, tag="qk",
                                      name=f"qk{npass}_{p}_{c}_{ps0}b")]
                    cuts = set(range(0, pw, 512)) | {pw}
                    qb = 1024 - (qg0 + ps0)      # qT tile boundary, local
                    if 0 < qb < pw:
                        cuts.add(qb)
                    cuts = sorted(cuts)
                    for s, e in zip(cuts[:-1], cuts[1:]):
                        gq = qg0 + ps0 + s
                        qt, qc = (box['qT_a'], gq) if gq < 1024 else (qT_b, gq - 1024)
                        for par, r0 in ((0, 0), (1, 64)):
                            nc.tensor.matmul(
                                tiles[par][:, s:e],
                                kT[r0:r0 + 64, p, c * P:(c + 1) * P],
                                qt[r0:r0 + 64, p, qc:qc + (e - s)],
                                start=True, stop=True)
                    for par in (0, 1):
                        nc.scalar.activation(
                            ehs[:, par, off + ps0:off + ps0 + pw],
                            tiles[par][:, 0:pw],
                            mybir.ActivationFunctionType.Exp, scale=0.125)
                    if ps0 == 0 and (npass == 1 or c >= QS):
                        for par in (0, 1):
                            nc.vector.tensor_mul(
                                out=ehs[:, par, off:off + P],
                                in0=ehs[:, par, off:off + P], in1=msk[:])

            def av_slot(ehs, p, j, npass):
                """AV + softmax-normalize for q-block j of pair p."""
                yn = box['ynp'].tile([P, 2, DH], BF16, tag="yn")
                for par in (0, 1):
                    yat = sps.tile([P, DH + 1], F32, tag="ya")
                    ya = yat[:]
                    nch = j + 1
                    for c in range(nch):
                        if npass == 1:
                            col = OFF1[c] + (j - c) * P
                        else:
                            col = OFF2[c] + (j - max(c, QS)) * P
                        nc.tensor.matmul(
                            ya, ehs[:, par, col:col + P],
                            vA[:, c, 2 * p + par, :],
                            start=(c == 0), stop=(c == nch - 1))
                    rec = box['ynp'].tile([P, 1], F32, tag="rec")
                    nc.vector.reciprocal(rec[:], ya[:, DH:DH + 1])
                    nc.vector.tensor_scalar_mul(yn[:, par, :], ya[:, 0:DH], rec[:])
                nc.sync.dma_start(yT[:, p, j * P:(j + 1) * P],
                                  yn.rearrange("p a b -> p (a b)"), transpose=True)

            def out_unit(tb, ehalf):
                """Partial output projection for t-block tb, 512 e-columns."""
                ps = pjp.tile([P, 512], F32, tag="pj")
                for kc in range(4):
                    nc.tensor.matmul(
                        ps[:], yT[:, kc, tb * P:(tb + 1) * P],
                        wo_s[:, kc, ehalf * 512:(ehalf + 1) * 512],
                        start=(kc == 0), stop=(kc == 3))
                ob = box['obp'].tile([P, 512], F32, tag="ob")
                nc.vector.tensor_copy(out=ob[:], in_=ps[:])
                nc.sync.dma_start(out[tb * P:(tb + 1) * P,
                                      ehalf * 512:(ehalf + 1) * 512], ob[:])

            xr = xT.rearrange("(dc p) t -> p dc t", p=P)
            with tc.tile_pool(name="long", bufs=1) as lg:
                # pair-3's pass-1 exp buffer outlives pass 1: its AV runs as
                # the pass-2 warm block
                eh1_sp = lg.tile([P, 2, OFF1[QS]], BF16)

                # ---------------- pass 1 --------------------------------------
                with (
                    tc.tile_pool(name="p1", bufs=1) as p1,
                    tc.tile_pool(name="ynp1", bufs=3) as ynp1,
                    tc.tile_pool(name="wsp", bufs=2) as wsp,
                    tc.tile_pool(name="eh1", bufs=2) as e1p,
                ):
                    box['ynp'] = ynp1
                    xTs_a = p1.tile([P, 8, 1024], BF16)
                    xTs_b = p1.tile([P, 8, 1024], BF16)
                    wv_s = p1.tile([P, 8, 512], BF16)
                    qT_a = p1.tile([P, NPAIR, 1024], BF16)
                    box['xTs_a'] = xTs_a
                    box['xTs_b'] = xTs_b
                    box['wv_s'] = wv_s
                    box['qT_a'] = qT_a
                    wk_t = {}
                    wq_t = {}

                    def stage_w(p):
                        wk_t[p] = wsp.tile([P, 8, P], BF16, tag="wk",
                                           name=f"wk_t{p}")
                        wq_t[p] = wsp.tile([P, 8, P], BF16, tag="wq",
                                           name=f"wq_t{p}")
                        nc.sync.dma_start(
                            wk_t[p][:], wk[:, p * P:(p + 1) * P].rearrange(
                                "(dc p) e -> p dc e", p=P))
                        nc.sync.dma_start(
                            wq_t[p][:], wq[:, p * P:(p + 1) * P].rearrange(
                                "(dc p) e -> p dc e", p=P))

                    # DMA order = first-use order: pair-0 weights + first x piece
                    # first, so the PE stream starts as early as possible
                    stage_w(0)
                    for nt in range(2):
                        nc.sync.dma_start(xTs_a[:, :, nt * 512:(nt + 1) * 512],
                                          xr[:, :, nt * 512:(nt + 1) * 512])
                    nc.sync.dma_start(xTs_b[:, :, 0:512], xr[:, :, 1024:1536])
                    nc.sync.dma_start(msk[:], mask[:])
                    nc.sync.dma_start(
                        wv_s[:], wv.rearrange("(dc p) e -> p dc e", p=P))
                    stage_w(1)
                    nc.sync.dma_start(xTs_b[:, :, 512:1024], xr[:, :, 1536:2048])
                    for kc in range(4):
                        nc.sync.dma_start(wo_s[:, kc, :],
                                          w_out[kc * P:(kc + 1) * P, :])

                    def k_unit(pr, half, tp):
                        xs = xTs_a if half == 0 else xTs_b
                        return lambda: proj_kq(
                            wk_t[pr],
                            kT[:, pr, half * 1024 + tp * 512:half * 1024 + (tp + 1) * 512],
                            xs, tp * 512)

                    def qa_unit(pr, tp):
                        return lambda: proj_kq(
                            wq_t[pr], qT_a[:, pr, tp * 512:(tp + 1) * 512],
                            xTs_a, tp * 512)

                    def qb_unit(pr, tp):
                        return lambda: proj_kq(
                            wq_t[pr], qT_b[:, pr, tp * 512:(tp + 1) * 512],
                            xTs_b, tp * 512)

                    def p1_units():
                        # all 8 K/Q units of a pair together, so its staged
                        # weight tiles are fully consumed within one section
                        # (the staging pool rotates with bufs=2)
                        def pre(pr):
                            yield k_unit(pr, 0, 0)
                            yield k_unit(pr, 0, 1)
                            yield k_unit(pr, 1, 0)
                            yield k_unit(pr, 1, 1)
                            yield qa_unit(pr, 0)
                            yield qa_unit(pr, 1)
                            yield qb_unit(pr, 0)
                            yield qb_unit(pr, 1)
                        yield from pre(0)
                        # all of V chunks 0..QS-1 before the first pass-1 AV
                        for tt in range(QS):
                            yield lambda tt=tt: proj_v(tt)
                        for pr in range(1, NPAIR):
                            yield from pre(pr)
                        # V chunks QS..15 (only needed by pass-2 AV)
                        for tt in range(QS, NCH):
                            yield lambda tt=tt: proj_v(tt)

                    gen = p1_units()

                    def fill(n):
                        for _ in range(n):
                            u = next(gen, None)
                            if u is None:
                                return
                            u()

                    # pass-1 pipeline: pair p's AV runs as filler inside pair
                    # p+1's QK chunk loop (one q-block per chunk)
                    fill(8)
                    eh1 = {}
                    for p in range(NPAIR):
                        if p + 2 < NPAIR:
                            stage_w(p + 2)
                        if p == NPAIR - 1:
                            eh1[p] = eh1_sp
                        else:
                            eh1[p] = e1p.tile([P, 2, OFF1[QS]], BF16, tag="eh1",
                                              name=f"eh1_{p}")
                        for c in range(QS):
                            qk_chunk(eh1[p], p, c, 1)
                            if p == 0 or c % 2 == 0:
                                fill(1)
                            if p > 0:
                                av_slot(eh1[p - 1], p - 1, c, 1)
                        if p < NPAIR - 1:
                            fill(8)
                    fill(100)

                # ---------------- pass 2 --------------------------------------
                with (
                    tc.tile_pool(name="eh2", bufs=2) as e2p,
                    tc.tile_pool(name="ynp2", bufs=3) as ynp2,
                    tc.tile_pool(name="obp2", bufs=2) as obp2,
                ):
                    box['ynp'] = ynp2
                    box['obp'] = obp2

                    # dense warm block: pair-3's deferred pass-1 AV plus the
                    # first output-projection units - a gap-free PE stretch so
                    # the HAM clock gate re-reaches 8/8 right after the pass
                    # boundary (it only re-warms after a ~3.4us gapless run)
                    for j in range(6):
                        av_slot(eh1_sp, NPAIR - 1, j, 1)
                    for k, (j, tb) in enumerate(zip(range(6, QS), range(4))):
                        av_slot(eh1_sp, NPAIR - 1, j, 1)
                        out_unit(tb, 0)
                        out_unit(tb, 1)

                    # filler thunks per pair, consumed between QK chunks
                    def mk_fillers(p):
                        f = []
                        if p == 0:
                            f += [lambda tb=tb: out_unit(tb, 0) for tb in range(4, QS)]
                        else:
                            f += [lambda j=j, p=p: av_slot(eh2[p - 1], p - 1, j, 2)
                                  for j in range(QS, 16)]
                            if p == 1:
                                f += [lambda tb=tb: out_unit(tb, 1) for tb in range(4, QS)]
                        return f

                    eh2 = {}
                    for p in range(NPAIR):
                        eh2[p] = e2p.tile([P, 2, OFF2[NCH]], BF16, tag="eh2",
                                          name=f"eh2_{p}")
                        fillers = mk_fillers(p)
                        fi = 0
                        for c in range(NCH):
                            qk_chunk(eh2[p], p, c, 2)
                            want = ((c + 1) * len(fillers)) // NCH
                            while fi < want:
                                fillers[fi]()
                                fi += 1
                        while fi < len(fillers):
                            fillers[fi]()
                            fi += 1
                    # tail: last pair AV, with the output projection lagging two
                    # q-blocks so it never waits on the yT transpose DMA
                    pend = []
                    for j in range(QS, 16):
                        av_slot(eh2[NPAIR - 1], NPAIR - 1, j, 2)
                        pend.append(j)
                        if len(pend) > 2:
                            jj = pend.pop(0)
                            out_unit(jj, 0)
                            out_unit(jj, 1)
                    for jj in pend:
                        out_unit(jj, 0)
                        out_unit(jj, 1)
                    if DEBUG:
                        dkT = nc.declare_dram_parameter(
                            "dbg_kT", [P, NPAIR * T], BF16, isOutput=True)
                        dqT = nc.declare_dram_parameter(
                            "dbg_qTb", [P, NPAIR * 1024], BF16, isOutput=True)
                        dvA = nc.declare_dram_parameter(
                            "dbg_vA", [P, NCH * 8 * (DH + 1)], BF16, isOutput=True)
                        dyT = nc.declare_dram_parameter(
                            "dbg_yT", [P, NPAIR * T], BF16, isOutput=True)
                        nc.sync.dma_start(dkT[:], kT.rearrange("p a b -> p (a b)"))
                        nc.sync.dma_start(dqT[:], qT_b.rearrange("p a b -> p (a b)"))
                        nc.sync.dma_start(dvA[:], vA.rearrange("p a b c -> p (a b c)"))
                        nc.sync.dma_start(dyT[:], yT.rearrange("p a b -> p (a b)"))

    _split_waits(nc, 1)
    return nc


def _split_waits(nc, maxw=1):
    """walrus rejects instructions with more than one sync wait; hoist extra
    waits onto preceding same-engine Drain instructions."""
    nsplit = 0
    for f in nc.m.functions:
        for b in f.blocks:
            insts = b.instructions
            new = []
            changed = False
            for inst in insts:
                si = inst.sync_info
                if si is not None and len(si.on_wait) > maxw:
                    waits = list(si.on_wait)
                    chunks = [waits[i:i + maxw] for i in range(0, len(waits), maxw)]
                    for ci, ch in enumerate(chunks[:-1]):
                        d = mybir.InstDrain(name=f"{inst.name}-wsplit{ci}", ins=[], outs=[])
                        d.engine = inst.engine
                        d.sync_info = mybir.SyncInfo(on_wait=ch, on_update=[])
                        new.append(d)
                        nsplit += 1
                    inst.sync_info = mybir.SyncInfo(
                        on_wait=chunks[-1], on_update=list(si.on_update))
                    changed = True
                new.append(inst)
            if changed:
                b.instructions = new
    return nsplit


def kernel(x, w_qkv, w_out):
    global _CACHED_NC, LAST_RESULTS
    x = np.asarray(x)
    w_qkv = np.asarray(w_qkv)
    w_out = np.asarray(w_out)
    B = x.shape[0]
    assert x.shape == (B, T, D) and B * 2 == 8

    if _CACHED_NC is None:
        _CACHED_NC = _build_nc()
    nc = _CACHED_NC

    wb = w_qkv.astype(ml_dtypes.bfloat16)
    wob = w_out.astype(ml_dtypes.bfloat16)
    # eh layout is [key-partition, query-free]: keep k <= q (upper triangle)
    mask = np.triu(np.ones((P, P), np.float32)).astype(ml_dtypes.bfloat16)

    xTb = [np.ascontiguousarray(x[b].T).astype(ml_dtypes.bfloat16)
           for b in range(B)]
    in_maps = []
    for core in range(8):
        b, r = divmod(core, 2)
        in_maps.append({
            "xT": xTb[b],
            "wq": np.ascontiguousarray(wb[:, r * 512:(r + 1) * 512]),
            "wk": np.ascontiguousarray(wb[:, D + r * 512:D + (r + 1) * 512]),
            "wv": np.ascontiguousarray(wb[:, 2 * D + r * 512:2 * D + (r + 1) * 512]),
            "w_out": np.ascontiguousarray(wob[r * 512:(r + 1) * 512, :]),
            "mask": mask,
        })

    res = run_bass_kernel_spmd(nc, in_maps, core_ids=list(range(8)))
    LAST_RESULTS = res

    y = np.empty((B, T, D), np.float32)
    for b in range(B):
        y[b] = res.results[2 * b]["out"]
        y[b] += res.results[2 * b + 1]["out"]
    return y
